# revision 39
# baseline (speedup 1.0000x reference)
"""CALoraLinear kernel for 8 TRN2 NeuronCores (Bass/Tile, SPMD).

Math (derived from the reference):
  orig = x @ W.T + bias
  top2 classes c1,c2 per row from pseudo_index[b, :64]
  g_j = <lora_A[c_j], x[b]>          (only rows 0..63 of lora_A are reachable)
  lora_out[b,o] = 16 * sum_c mask[b,c] * G[b,c] * lora_B[o,c]
  out = orig + lora_out + bias       (bias added twice)

Sharding: column-shard W across the 8 cores (each core owns 512 output
columns, full batch); x / lora_A / pseudo_index replicated. Host
concatenates the per-core [512, 512] blocks along the output axis.

Main/G matmuls stream float16 operands (the PE upconverts 2-byte floats
to FP22 internally, so fp16 matches float32r multiply precision at half
the DMA bytes; measured ~3e-4 rel err). The LoRA tail matmul runs as
float32r. Operands stream through interleaved per-K-chunk DRAM buffers
(one DMA per chunk, alternating between the two HWDGE rings), with the
PE start gated on a buffered chunk backlog to keep the HAM warm.
"""

import os
import sys

for _p in ("/opt/trn_rl_repo",):
    if _p not in sys.path:
        sys.path.insert(0, _p)

import numpy as np

import concourse.bass as bass
import concourse.bacc as bacc
import concourse.mybir as mybir
from concourse.tile import TileContext, add_dep_helper
from concourse.bass_utils import run_bass_kernel_spmd


def _ensure_ntff_hook_module():
    """run_bass_kernel_spmd(trace=True) imports antenv.axon_hooks, which the
    agent image's antenv package lacks. Provide it (and register the real
    ctypes NTFF hook when available) so a tracing caller doesn't crash."""
    import types

    try:
        import antenv
    except ImportError:
        return
    if getattr(antenv, "axon_hooks", None) is not None:
        return
    mod = types.ModuleType("antenv.axon_hooks")
    state = {"hook": None}
    mod.set_axon_ntff_profile_hook = lambda h: state.__setitem__("hook", h)
    mod.get_axon_ntff_profile_hook = lambda: state["hook"]
    sys.modules["antenv.axon_hooks"] = mod
    antenv.axon_hooks = mod
    try:
        from trn_agent_boot.trn_boot import _ntff_profile_via_ctypes

        mod.set_axon_ntff_profile_hook(
            _ntff_profile_via_ctypes("/opt/axon/libaxon_pjrt.so")
        )
    except Exception:
        pass


_ensure_ntff_hook_module()

B, IN, OUT = 512, 4096, 4096
NUM_CLASS, RANK = 64, 8
NCORES = 8
OUT_L = OUT // NCORES  # 512
P = 128
KT = IN // P           # 32 k-tiles
BT = B // P            # 4 batch tiles
CHUNK = 2              # k-tiles per DMA chunk
NCHUNK = KT // CHUNK   # 16

# column layout of one xw chunk: [x: CHUNK*B][w: CHUNK*OUT_L][a: CHUNK*64]
XOFF = 0
WOFF = CHUNK * B
AOFF = WOFF + CHUNK * OUT_L
WIDTH = AOFF + CHUNK * NUM_CLASS

# pp layout: [ps: BT*64][psT: B][bS: OUT_L (rows 0:65)]
PSOFF = 0
PTOFF = BT * NUM_CLASS
BSOFF = PTOFF + B
PPW = BSOFF + OUT_L

F32 = mybir.dt.float32
F32R = mybir.dt.float32r
BF16 = mybir.dt.bfloat16
F16 = mybir.dt.float16
X = mybir.AxisListType.X

# Stream dtype for the big matmul operands. The PE upconverts 2-byte floats
# to FP22 internally, so fp16 matches float32r's multiply precision while
# halving DMA bytes; only the fp16 input rounding (2^-11) adds error.
#   f32r: ~1.6e-4 rel err, DMA-bound (~77us)
#   f16:  ~5e-4 rel err, PE-bound (~65us)   <- default
#   bf16: ~2.4e-3 rel err (no reason to use; f16 is same speed)
_KDT = os.environ.get("KDT", "f16")
SDT = {"f16": F16, "bf16": BF16, "f32r": F32R}[_KDT]
STREAM_2B = SDT in (F16, BF16)

_cache = {}
# test.py reads this after a traced run for HW exec time
last_results = None


def _build():
    key = f"nc_{_KDT}"
    if key in _cache:
        return _cache[key]
    nc = bacc.Bacc(
        bass.get_trn_type() or "TRN2",
        target_bir_lowering=False,
        debug=False,
        num_devices=NCORES,
    )

    xw = nc.dram_tensor("xw", [NCHUNK, P, WIDTH], SDT, kind="ExternalInput")
    pp = nc.dram_tensor("pp", [P, PPW], F32R, kind="ExternalInput")
    out = nc.dram_tensor("out", [B, OUT_L], F32, kind="ExternalOutput")

    with TileContext(nc) as tc:
        with (
            tc.tile_pool(name="xwp", bufs=1) as xwpool,
            tc.tile_pool(name="sml", bufs=1) as spool,
            tc.tile_pool(name="tl", bufs=1) as tpool,
            tc.tile_pool(name="op", bufs=1) as opool,
            tc.tile_pool(name="dr", bufs=1, space="DRAM") as dpool,
            tc.tile_pool(name="ps", bufs=1, space="PSUM") as ppool,
        ):
            # ---- small inputs (one ACT-ring DMA) ----
            pp_sb = spool.tile([P, PPW], F32R)
            nc.scalar.dma_start(out=pp_sb, in_=pp[:, :])
            ps_sb = pp_sb[:, PSOFF : PSOFF + BT * NUM_CLASS].bitcast(F32)
            psT_sb = pp_sb[:NUM_CLASS, PTOFF : PTOFF + B].bitcast(F32)
            bS_sb = pp_sb[: NUM_CLASS + 1, BSOFF : BSOFF + OUT_L]

            # ---- top-2 threshold per batch row (DVE, alongside matmuls) ----
            m2col = spool.tile([P, BT], F32)
            for bt in range(BT):
                pt = ps_sb[:, bt * NUM_CLASS : (bt + 1) * NUM_CLASS]
                m1 = spool.tile([P, 1], F32, tag=f"m1_{bt}")
                nc.vector.reduce_max(out=m1, in_=pt, axis=X)
                negmask = spool.tile([P, NUM_CLASS], F32, tag=f"nm_{bt}")
                # (pt >= m1) * -1e30  -> additive mask that kills the max
                nc.vector.tensor_scalar(
                    out=negmask,
                    in0=pt,
                    scalar1=m1,
                    scalar2=-1.0e30,
                    op0=mybir.AluOpType.is_ge,
                    op1=mybir.AluOpType.mult,
                )
                p2 = spool.tile([P, NUM_CLASS], F32, tag=f"p2_{bt}")
                nc.vector.tensor_tensor(
                    out=p2, in0=pt, in1=negmask, op=mybir.AluOpType.add
                )
                nc.vector.reduce_max(out=m2col[:, bt : bt + 1], in_=p2, axis=X)

            # ---- PSUM accumulators ----
            mps = [
                ppool.tile([P, OUT_L], F32, tag=f"main{bt}", name=f"main{bt}")
                for bt in range(BT)
            ]
            # bf16: G accumulates as two concurrent column-tiles of one PSUM
            # bank (even k -> rows 0:64, odd k -> rows 64:128); the fp32r
            # matmul path does not support tile_position, so it runs unpacked.
            G_PACK = STREAM_2B
            gt_rows = 2 * NUM_CLASS if G_PACK else NUM_CLASS
            gt_ps = ppool.tile([gt_rows, B], F32, tag="gt", name="gt_ps")

            # ---- PE warm-up: dummy matmuls on scratch SBUF during the DMA
            # ramp-in so the HAM clock-gate is at K=8/8 (2.4 GHz) when the
            # real stream starts (idle PE re-throttles to half rate) ----
            warm = spool.tile([P, OUT_L], F16, name="warm")
            nc.vector.memset(warm, 1.0)
            wps = ppool.tile([P, OUT_L], F32, tag="warm", name="wps")
            for _ in range(44):
                nc.tensor.matmul(
                    wps, lhsT=warm[:, 0:P], rhs=warm, start=True, stop=True
                )

            # ---- main streaming loop over K chunks (one DMA per chunk,
            # alternating HWDGE rings) ----
            # 2-byte streams are PE-bound: start the PE sooner
            GATE = 1 if STREAM_2B else 3
            first_mm = None
            for c in range(NCHUNK):
                xwc = xwpool.tile([P, WIDTH], SDT, tag=f"xwc{c}", name=f"xwc{c}")
                dma_eng = nc.sync if c % 2 == 0 else nc.scalar
                xwc_dma = dma_eng.dma_start(out=xwc, in_=xw[c])
                if c == GATE and first_mm is not None:
                    add_dep_helper(
                        first_mm.ins, xwc_dma.ins,
                        reason="gate PE start on a buffered chunk backlog",
                    )
                for kk in range(CHUNK):
                    k = c * CHUNK + kk
                    xk = xwc[:, XOFF + kk * B : XOFF + (kk + 1) * B]
                    wk = xwc[:, WOFF + kk * OUT_L : WOFF + (kk + 1) * OUT_L]
                    ak = xwc[:, AOFF + kk * NUM_CLASS : AOFF + (kk + 1) * NUM_CLASS]
                    def _mains(xk=xk, wk=wk, k=k):
                        global_first = None
                        for bt in range(BT):
                            mm = nc.tensor.matmul(
                                mps[bt],
                                lhsT=xk[:, bt * P : (bt + 1) * P],
                                rhs=wk,
                                start=(k == 0),
                                stop=False,
                            )
                            if global_first is None:
                                global_first = mm
                        return global_first
                    if k < KT - 2:
                        mm = _mains()
                        if first_mm is None:
                            first_mm = mm
                    if G_PACK:
                        half = k % 2
                        nc.tensor.matmul(
                            gt_ps[half * NUM_CLASS : (half + 1) * NUM_CLASS, :],
                            lhsT=ak,
                            rhs=xk,
                            start=(k == half),
                            stop=(k == KT - 2 + half),
                            tile_position=(0, half * NUM_CLASS),
                        )
                    else:
                        nc.tensor.matmul(
                            gt_ps,
                            lhsT=ak,
                            rhs=xk,
                            start=(k == 0),
                            stop=(k == KT - 1),
                        )
                    if k >= KT - 2:
                        # last two k: G first (above), mains after, so the
                        # DVE lora chain overlaps the final main matmuls
                        _mains()
            # ---- LoRA tail ----
            # threshold shuffle on the GPSIMD (SWDGE) path: concurrent with
            # both HWDGE rings, so nothing queues behind the chunk stream.
            # partition->free [128, BT] -> flat [B] via a DRAM bounce, then
            # broadcast-read across 64 partitions (step-0 source dim).
            m2d = dpool.tile([BT, P], F32)
            nc.gpsimd.dma_start(out=m2d.rearrange("bt p -> p bt"), in_=m2col[:, :])
            thr_sb = spool.tile([NUM_CLASS, B], F32)
            nc.gpsimd.dma_start(
                out=thr_sb,
                in_=m2d.rearrange("bt p -> (bt p)")[None, :].broadcast_to(
                    [NUM_CLASS, B]
                ),
            )
            thr2 = tpool.tile([NUM_CLASS, B], F32)
            nc.vector.tensor_copy(out=thr2, in_=thr_sb)
            psT2 = tpool.tile([NUM_CLASS, B], F32)
            nc.vector.tensor_copy(out=psT2, in_=psT_sb)
            maskT = tpool.tile([NUM_CLASS, B], F32)
            nc.vector.tensor_tensor(
                out=maskT, in0=psT2, in1=thr2, op=mybir.AluOpType.is_ge
            )
            gts = tpool.tile([NUM_CLASS, B], F32)
            if G_PACK:
                gt0 = tpool.tile([NUM_CLASS, B], F32)
                nc.vector.tensor_copy(out=gt0, in_=gt_ps[0:NUM_CLASS, :])
                nc.vector.tensor_tensor(
                    out=gts,
                    in0=gt0,
                    in1=gt_ps[NUM_CLASS : 2 * NUM_CLASS, :],
                    op=mybir.AluOpType.add,
                )
            else:
                nc.vector.tensor_copy(out=gts, in_=gt_ps)
            ht = tpool.tile([NUM_CLASS + 1, B], F32R)
            # ones row via in0*0+1 (Memset can't write float32r)
            nc.vector.tensor_scalar(
                out=ht[NUM_CLASS : NUM_CLASS + 1, :],
                in0=thr2[0:1, :],
                scalar1=0.0,
                scalar2=1.0,
                op0=mybir.AluOpType.mult,
                op1=mybir.AluOpType.add,
            )
            nc.vector.tensor_tensor(
                out=ht[0:NUM_CLASS, :], in0=gts, in1=maskT,
                op=mybir.AluOpType.mult,
            )
            for bt in range(BT):
                nc.tensor.matmul(
                    mps[bt],
                    lhsT=ht[:, bt * P : (bt + 1) * P],
                    rhs=bS_sb,
                    start=False,
                    stop=True,
                )

            # ---- epilogue: PSUM -> SBUF -> DRAM, pipelined per tile ----
            o_all = opool.tile([P, BT * OUT_L], F32)
            for bt in range(BT):
                nc.vector.tensor_copy(
                    out=o_all[:, bt * OUT_L : (bt + 1) * OUT_L], in_=mps[bt]
                )
                nc.sync.dma_start(
                    out=out[bt * P : (bt + 1) * P, :],
                    in_=o_all[:, bt * OUT_L : (bt + 1) * OUT_L],
                )

    nc.finalize()
    _cache[key] = nc
    return nc


def _pack_inputs(x, pseudo_index, weight, bias, lora_A, lora_B):
    """Build the interleaved per-core xw buffers + replicated small inputs."""
    xT = np.ascontiguousarray(x.T)                   # [IN, B]
    aT = np.ascontiguousarray(lora_A[:NUM_CLASS].T)  # [IN, 64]

    # [c, kk, p, d] -> [c, p, kk, d]
    x4 = xT.reshape(NCHUNK, CHUNK, P, B).transpose(0, 2, 1, 3)
    a4 = aT.reshape(NCHUNK, CHUNK, P, NUM_CLASS).transpose(0, 2, 1, 3)

    pp_base = np.zeros((P, PPW), dtype=np.float32)
    pp_base[:, PSOFF : PSOFF + BT * NUM_CLASS] = (
        pseudo_index.reshape(BT, P, NUM_CLASS)
        .transpose(1, 0, 2)
        .reshape(P, BT * NUM_CLASS)
    )
    pp_base[:NUM_CLASS, PTOFF : PTOFF + B] = pseudo_index.T

    in_maps = []
    for i in range(NCORES):
        o0 = i * OUT_L
        wTi = weight[o0 : o0 + OUT_L].T              # [IN, OUT_L] (view)
        w4 = wTi.reshape(NCHUNK, CHUNK, P, OUT_L).transpose(0, 2, 1, 3)
        if SDT == F16:
            np_sdt = np.float16
        elif SDT == BF16:
            import ml_dtypes

            np_sdt = ml_dtypes.bfloat16
        else:
            np_sdt = np.float32
        xwi = np.empty((NCHUNK, P, WIDTH), dtype=np_sdt)
        xwi[:, :, XOFF:WOFF] = x4.reshape(NCHUNK, P, CHUNK * B)
        xwi[:, :, WOFF:AOFF] = w4.reshape(NCHUNK, P, CHUNK * OUT_L)
        xwi[:, :, AOFF:WIDTH] = a4.reshape(NCHUNK, P, CHUNK * NUM_CLASS)
        ppi = pp_base.copy()
        ppi[:NUM_CLASS, BSOFF : BSOFF + OUT_L] = (
            16.0 * lora_B[o0 : o0 + OUT_L, :NUM_CLASS].T
        )
        ppi[NUM_CLASS, BSOFF : BSOFF + OUT_L] = 2.0 * bias[o0 : o0 + OUT_L]
        in_maps.append({"xw": xwi, "pp": ppi})
    return in_maps


def kernel(x, pseudo_index, weight, bias, lora_A, lora_B):
    global last_results
    x = np.ascontiguousarray(np.asarray(x, dtype=np.float32))
    pseudo_index = np.ascontiguousarray(np.asarray(pseudo_index, dtype=np.float32))
    weight = np.asarray(weight, dtype=np.float32)
    bias = np.asarray(bias, dtype=np.float32)
    lora_A = np.asarray(lora_A, dtype=np.float32)
    lora_B = np.asarray(lora_B, dtype=np.float32)

    nc = _build()
    in_maps = _pack_inputs(x, pseudo_index, weight, bias, lora_A, lora_B)
    res = run_bass_kernel_spmd(nc, in_maps, list(range(NCORES)))
    last_results = res
    return np.hstack([res.results[i]["out"] for i in range(NCORES)])


# revision 40
# speedup vs baseline: 1.2162x; 1.2162x over previous
"""CALoraLinear kernel for 8 TRN2 NeuronCores (Bass/Tile, SPMD).

Math (derived from the reference):
  orig = x @ W.T + bias
  top2 classes c1,c2 per row from pseudo_index[b, :64]
  g_j = <lora_A[c_j], x[b]>          (only rows 0..63 of lora_A are reachable)
  lora_out[b,o] = 16 * sum_c mask[b,c] * G[b,c] * lora_B[o,c]
  out = orig + lora_out + bias       (bias added twice)

Sharding: column-shard W across the 8 cores (each core owns 512 output
columns, full batch); x / lora_A / pseudo_index replicated. Host
concatenates the per-core [512, 512] blocks along the output axis.

Main/G matmuls stream float16 operands (the PE upconverts 2-byte floats
to FP22 internally, so fp16 matches float32r multiply precision at half
the DMA bytes; measured ~3e-4 rel err). The LoRA tail matmul runs as
float32r. Operands stream through interleaved per-K-chunk DRAM buffers
(one DMA per chunk, alternating between the two HWDGE rings), with the
PE start gated on a buffered chunk backlog to keep the HAM warm.
"""

import os
import sys

for _p in ("/opt/trn_rl_repo",):
    if _p not in sys.path:
        sys.path.insert(0, _p)

import numpy as np

import concourse.bass as bass
import concourse.bacc as bacc
import concourse.mybir as mybir
from concourse.tile import TileContext, add_dep_helper
from concourse.bass_utils import run_bass_kernel_spmd


def _ensure_ntff_hook_module():
    """run_bass_kernel_spmd(trace=True) imports antenv.axon_hooks, which the
    agent image's antenv package lacks. Provide it (and register the real
    ctypes NTFF hook when available) so a tracing caller doesn't crash."""
    import types

    try:
        import antenv
    except ImportError:
        return
    if getattr(antenv, "axon_hooks", None) is not None:
        return
    mod = types.ModuleType("antenv.axon_hooks")
    state = {"hook": None}
    mod.set_axon_ntff_profile_hook = lambda h: state.__setitem__("hook", h)
    mod.get_axon_ntff_profile_hook = lambda: state["hook"]
    sys.modules["antenv.axon_hooks"] = mod
    antenv.axon_hooks = mod
    try:
        from trn_agent_boot.trn_boot import _ntff_profile_via_ctypes

        mod.set_axon_ntff_profile_hook(
            _ntff_profile_via_ctypes("/opt/axon/libaxon_pjrt.so")
        )
    except Exception:
        pass


_ensure_ntff_hook_module()

B, IN, OUT = 512, 4096, 4096
NUM_CLASS, RANK = 64, 8
NCORES = 8
OUT_L = OUT // NCORES  # 512
P = 128
KT = IN // P           # 32 k-tiles
BT = B // P            # 4 batch tiles
CHUNK = 2              # k-tiles per DMA chunk
NCHUNK = KT // CHUNK   # 16

# column layout of one xw chunk: [x: CHUNK*B][w: CHUNK*OUT_L][a: CHUNK*64]
XOFF = 0
WOFF = CHUNK * B
AOFF = WOFF + CHUNK * OUT_L
WIDTH = AOFF + CHUNK * NUM_CLASS

# pp layout: [ps: BT*64][psT: B][bS: OUT_L (rows 0:65)]
PSOFF = 0
PTOFF = BT * NUM_CLASS
BSOFF = PTOFF + B
PPW = BSOFF + OUT_L

F32 = mybir.dt.float32
F32R = mybir.dt.float32r
BF16 = mybir.dt.bfloat16
F16 = mybir.dt.float16
X = mybir.AxisListType.X

# Stream dtype for the big matmul operands. The PE upconverts 2-byte floats
# to FP22 internally, so fp16 matches float32r's multiply precision while
# halving DMA bytes; only the fp16 input rounding (2^-11) adds error.
#   f32r: ~1.6e-4 rel err, DMA-bound (~77us)
#   f16:  ~5e-4 rel err, PE-bound (~65us)   <- default
#   bf16: ~2.4e-3 rel err (no reason to use; f16 is same speed)
_KDT = os.environ.get("KDT", "f16")
SDT = {"f16": F16, "bf16": BF16, "f32r": F32R}[_KDT]
STREAM_2B = SDT in (F16, BF16)

_cache = {}
# test.py reads this after a traced run for HW exec time
last_results = None


def _build():
    key = f"nc_{_KDT}"
    if key in _cache:
        return _cache[key]
    nc = bacc.Bacc(
        bass.get_trn_type() or "TRN2",
        target_bir_lowering=False,
        debug=False,
        num_devices=NCORES,
    )

    xw = nc.dram_tensor("xw", [NCHUNK, P, WIDTH], SDT, kind="ExternalInput")
    pp = nc.dram_tensor("pp", [P, PPW], F32R, kind="ExternalInput")
    out = nc.dram_tensor("out", [B, OUT_L], F32, kind="ExternalOutput")

    with TileContext(nc) as tc:
        with (
            tc.tile_pool(name="xwp", bufs=1) as xwpool,
            tc.tile_pool(name="sml", bufs=1) as spool,
            tc.tile_pool(name="tl", bufs=1) as tpool,
            tc.tile_pool(name="op", bufs=1) as opool,
            tc.tile_pool(name="dr", bufs=1, space="DRAM") as dpool,
            tc.tile_pool(name="ps", bufs=1, space="PSUM") as ppool,
        ):
            # ---- small inputs (one ACT-ring DMA) ----
            pp_sb = spool.tile([P, PPW], F32R)
            nc.scalar.dma_start(out=pp_sb, in_=pp[:, :])
            ps_sb = pp_sb[:, PSOFF : PSOFF + BT * NUM_CLASS].bitcast(F32)
            psT_sb = pp_sb[:NUM_CLASS, PTOFF : PTOFF + B].bitcast(F32)
            bS_sb = pp_sb[: NUM_CLASS + 1, BSOFF : BSOFF + OUT_L]

            # ---- top-2 threshold per batch row (DVE, alongside matmuls) ----
            m2col = spool.tile([P, BT], F32)
            for bt in range(BT):
                pt = ps_sb[:, bt * NUM_CLASS : (bt + 1) * NUM_CLASS]
                m1 = spool.tile([P, 1], F32, tag=f"m1_{bt}")
                nc.vector.reduce_max(out=m1, in_=pt, axis=X)
                negmask = spool.tile([P, NUM_CLASS], F32, tag=f"nm_{bt}")
                # (pt >= m1) * -1e30  -> additive mask that kills the max
                nc.vector.tensor_scalar(
                    out=negmask,
                    in0=pt,
                    scalar1=m1,
                    scalar2=-1.0e30,
                    op0=mybir.AluOpType.is_ge,
                    op1=mybir.AluOpType.mult,
                )
                p2 = spool.tile([P, NUM_CLASS], F32, tag=f"p2_{bt}")
                nc.vector.tensor_tensor(
                    out=p2, in0=pt, in1=negmask, op=mybir.AluOpType.add
                )
                nc.vector.reduce_max(out=m2col[:, bt : bt + 1], in_=p2, axis=X)

            # ---- PSUM accumulators ----
            mps = [
                ppool.tile([P, OUT_L], F32, tag=f"main{bt}", name=f"main{bt}")
                for bt in range(BT)
            ]
            # bf16: G accumulates as two concurrent column-tiles of one PSUM
            # bank (even k -> rows 0:64, odd k -> rows 64:128); the fp32r
            # matmul path does not support tile_position, so it runs unpacked.
            G_PACK = STREAM_2B
            gt_rows = 2 * NUM_CLASS if G_PACK else NUM_CLASS
            gt_ps = ppool.tile([gt_rows, B], F32, tag="gt", name="gt_ps")

            # ---- main streaming loop over K chunks (one DMA per chunk,
            # alternating HWDGE rings) ----
            # 2-byte streams are PE-bound: start the PE sooner
            GATE = 1 if STREAM_2B else 3
            first_mm = None
            for c in range(NCHUNK):
                xwc = xwpool.tile([P, WIDTH], SDT, tag=f"xwc{c}", name=f"xwc{c}")
                dma_eng = nc.sync if c % 2 == 0 else nc.scalar
                xwc_dma = dma_eng.dma_start(out=xwc, in_=xw[c])
                if c == GATE and first_mm is not None:
                    add_dep_helper(
                        first_mm.ins, xwc_dma.ins,
                        reason="gate PE start on a buffered chunk backlog",
                    )
                for kk in range(CHUNK):
                    k = c * CHUNK + kk
                    xk = xwc[:, XOFF + kk * B : XOFF + (kk + 1) * B]
                    wk = xwc[:, WOFF + kk * OUT_L : WOFF + (kk + 1) * OUT_L]
                    ak = xwc[:, AOFF + kk * NUM_CLASS : AOFF + (kk + 1) * NUM_CLASS]
                    def _mains(xk=xk, wk=wk, k=k):
                        global_first = None
                        for bt in range(BT):
                            mm = nc.tensor.matmul(
                                mps[bt],
                                lhsT=xk[:, bt * P : (bt + 1) * P],
                                rhs=wk,
                                start=(k == 0),
                                stop=False,
                            )
                            if global_first is None:
                                global_first = mm
                        return global_first
                    if k < KT - 2:
                        mm = _mains()
                        if first_mm is None:
                            first_mm = mm
                    if G_PACK:
                        half = k % 2
                        nc.tensor.matmul(
                            gt_ps[half * NUM_CLASS : (half + 1) * NUM_CLASS, :],
                            lhsT=ak,
                            rhs=xk,
                            start=(k == half),
                            stop=(k == KT - 2 + half),
                            tile_position=(0, half * NUM_CLASS),
                        )
                    else:
                        nc.tensor.matmul(
                            gt_ps,
                            lhsT=ak,
                            rhs=xk,
                            start=(k == 0),
                            stop=(k == KT - 1),
                        )
                    if k >= KT - 2:
                        # last two k: G first (above), mains after, so the
                        # DVE lora chain overlaps the final main matmuls
                        _mains()
            # ---- LoRA tail ----
            # threshold shuffle on the GPSIMD (SWDGE) path: concurrent with
            # both HWDGE rings, so nothing queues behind the chunk stream.
            # partition->free [128, BT] -> flat [B] via a DRAM bounce, then
            # broadcast-read across 64 partitions (step-0 source dim).
            m2d = dpool.tile([BT, P], F32)
            nc.gpsimd.dma_start(out=m2d.rearrange("bt p -> p bt"), in_=m2col[:, :])
            thr_sb = spool.tile([NUM_CLASS, B], F32)
            nc.gpsimd.dma_start(
                out=thr_sb,
                in_=m2d.rearrange("bt p -> (bt p)")[None, :].broadcast_to(
                    [NUM_CLASS, B]
                ),
            )
            thr2 = tpool.tile([NUM_CLASS, B], F32)
            nc.vector.tensor_copy(out=thr2, in_=thr_sb)
            psT2 = tpool.tile([NUM_CLASS, B], F32)
            nc.vector.tensor_copy(out=psT2, in_=psT_sb)
            maskT = tpool.tile([NUM_CLASS, B], F32)
            nc.vector.tensor_tensor(
                out=maskT, in0=psT2, in1=thr2, op=mybir.AluOpType.is_ge
            )
            gts = tpool.tile([NUM_CLASS, B], F32)
            if G_PACK:
                gt0 = tpool.tile([NUM_CLASS, B], F32)
                nc.vector.tensor_copy(out=gt0, in_=gt_ps[0:NUM_CLASS, :])
                nc.vector.tensor_tensor(
                    out=gts,
                    in0=gt0,
                    in1=gt_ps[NUM_CLASS : 2 * NUM_CLASS, :],
                    op=mybir.AluOpType.add,
                )
            else:
                nc.vector.tensor_copy(out=gts, in_=gt_ps)
            ht = tpool.tile([NUM_CLASS + 1, B], F32R)
            # ones row via in0*0+1 (Memset can't write float32r)
            nc.vector.tensor_scalar(
                out=ht[NUM_CLASS : NUM_CLASS + 1, :],
                in0=thr2[0:1, :],
                scalar1=0.0,
                scalar2=1.0,
                op0=mybir.AluOpType.mult,
                op1=mybir.AluOpType.add,
            )
            nc.vector.tensor_tensor(
                out=ht[0:NUM_CLASS, :], in0=gts, in1=maskT,
                op=mybir.AluOpType.mult,
            )
            for bt in range(BT):
                nc.tensor.matmul(
                    mps[bt],
                    lhsT=ht[:, bt * P : (bt + 1) * P],
                    rhs=bS_sb,
                    start=False,
                    stop=True,
                )

            # ---- epilogue: PSUM -> SBUF -> DRAM, pipelined per tile ----
            o_all = opool.tile([P, BT * OUT_L], F32)
            for bt in range(BT):
                nc.vector.tensor_copy(
                    out=o_all[:, bt * OUT_L : (bt + 1) * OUT_L], in_=mps[bt]
                )
                nc.sync.dma_start(
                    out=out[bt * P : (bt + 1) * P, :],
                    in_=o_all[:, bt * OUT_L : (bt + 1) * OUT_L],
                )

    nc.finalize()
    _cache[key] = nc
    return nc


def _pack_inputs(x, pseudo_index, weight, bias, lora_A, lora_B):
    """Build the interleaved per-core xw buffers + replicated small inputs."""
    xT = np.ascontiguousarray(x.T)                   # [IN, B]
    aT = np.ascontiguousarray(lora_A[:NUM_CLASS].T)  # [IN, 64]

    # [c, kk, p, d] -> [c, p, kk, d]
    x4 = xT.reshape(NCHUNK, CHUNK, P, B).transpose(0, 2, 1, 3)
    a4 = aT.reshape(NCHUNK, CHUNK, P, NUM_CLASS).transpose(0, 2, 1, 3)

    pp_base = np.zeros((P, PPW), dtype=np.float32)
    pp_base[:, PSOFF : PSOFF + BT * NUM_CLASS] = (
        pseudo_index.reshape(BT, P, NUM_CLASS)
        .transpose(1, 0, 2)
        .reshape(P, BT * NUM_CLASS)
    )
    pp_base[:NUM_CLASS, PTOFF : PTOFF + B] = pseudo_index.T

    in_maps = []
    for i in range(NCORES):
        o0 = i * OUT_L
        wTi = weight[o0 : o0 + OUT_L].T              # [IN, OUT_L] (view)
        w4 = wTi.reshape(NCHUNK, CHUNK, P, OUT_L).transpose(0, 2, 1, 3)
        if SDT == F16:
            np_sdt = np.float16
        elif SDT == BF16:
            import ml_dtypes

            np_sdt = ml_dtypes.bfloat16
        else:
            np_sdt = np.float32
        xwi = np.empty((NCHUNK, P, WIDTH), dtype=np_sdt)
        xwi[:, :, XOFF:WOFF] = x4.reshape(NCHUNK, P, CHUNK * B)
        xwi[:, :, WOFF:AOFF] = w4.reshape(NCHUNK, P, CHUNK * OUT_L)
        xwi[:, :, AOFF:WIDTH] = a4.reshape(NCHUNK, P, CHUNK * NUM_CLASS)
        ppi = pp_base.copy()
        ppi[:NUM_CLASS, BSOFF : BSOFF + OUT_L] = (
            16.0 * lora_B[o0 : o0 + OUT_L, :NUM_CLASS].T
        )
        ppi[NUM_CLASS, BSOFF : BSOFF + OUT_L] = 2.0 * bias[o0 : o0 + OUT_L]
        in_maps.append({"xw": xwi, "pp": ppi})
    return in_maps


def kernel(x, pseudo_index, weight, bias, lora_A, lora_B):
    global last_results
    x = np.ascontiguousarray(np.asarray(x, dtype=np.float32))
    pseudo_index = np.ascontiguousarray(np.asarray(pseudo_index, dtype=np.float32))
    weight = np.asarray(weight, dtype=np.float32)
    bias = np.asarray(bias, dtype=np.float32)
    lora_A = np.asarray(lora_A, dtype=np.float32)
    lora_B = np.asarray(lora_B, dtype=np.float32)

    nc = _build()
    in_maps = _pack_inputs(x, pseudo_index, weight, bias, lora_A, lora_B)
    res = run_bass_kernel_spmd(nc, in_maps, list(range(NCORES)))
    last_results = res
    return np.hstack([res.results[i]["out"] for i in range(NCORES)])


# revision 41
# speedup vs baseline: 1.2609x; 1.0368x over previous
"""CALoraLinear kernel for 8 TRN2 NeuronCores (Bass/Tile, SPMD).

Math (derived from the reference):
  orig = x @ W.T + bias
  top2 classes c1,c2 per row from pseudo_index[b, :64]
  g_j = <lora_A[c_j], x[b]>          (only rows 0..63 of lora_A are reachable)
  lora_out[b,o] = 16 * sum_c mask[b,c] * G[b,c] * lora_B[o,c]
  out = orig + lora_out + bias       (bias added twice)

Sharding: column-shard W across the 8 cores (each core owns 512 output
columns, full batch); x / lora_A / pseudo_index replicated. Host
concatenates the per-core [512, 512] blocks along the output axis.

Main/G matmuls stream float16 operands (the PE upconverts 2-byte floats
to FP22 internally, so fp16 matches float32r multiply precision at half
the DMA bytes; measured ~3e-4 rel err). The LoRA tail matmul runs as
float32r. Operands stream through interleaved per-K-chunk DRAM buffers
(one DMA per chunk, alternating between the two HWDGE rings), with the
PE start gated on a buffered chunk backlog to keep the HAM warm.
"""

import os
import sys

for _p in ("/opt/trn_rl_repo",):
    if _p not in sys.path:
        sys.path.insert(0, _p)

import numpy as np

import concourse.bass as bass
import concourse.bacc as bacc
import concourse.mybir as mybir
from concourse.tile import TileContext, add_dep_helper
from concourse.bass_utils import run_bass_kernel_spmd


def _ensure_ntff_hook_module():
    """run_bass_kernel_spmd(trace=True) imports antenv.axon_hooks, which the
    agent image's antenv package lacks. Provide it (and register the real
    ctypes NTFF hook when available) so a tracing caller doesn't crash."""
    import types

    try:
        import antenv
    except ImportError:
        return
    if getattr(antenv, "axon_hooks", None) is not None:
        return
    mod = types.ModuleType("antenv.axon_hooks")
    state = {"hook": None}
    mod.set_axon_ntff_profile_hook = lambda h: state.__setitem__("hook", h)
    mod.get_axon_ntff_profile_hook = lambda: state["hook"]
    sys.modules["antenv.axon_hooks"] = mod
    antenv.axon_hooks = mod
    try:
        from trn_agent_boot.trn_boot import _ntff_profile_via_ctypes

        mod.set_axon_ntff_profile_hook(
            _ntff_profile_via_ctypes("/opt/axon/libaxon_pjrt.so")
        )
    except Exception:
        pass


_ensure_ntff_hook_module()

B, IN, OUT = 512, 4096, 4096
NUM_CLASS, RANK = 64, 8
NCORES = 8
OUT_L = OUT // NCORES  # 512
P = 128
KT = IN // P           # 32 k-tiles
BT = B // P            # 4 batch tiles
CHUNK = 2              # k-tiles per DMA chunk
NCHUNK = KT // CHUNK   # 16

# column layout of one xw chunk: [x: CHUNK*B][w: CHUNK*OUT_L][a: CHUNK*64]
XOFF = 0
WOFF = CHUNK * B
AOFF = WOFF + CHUNK * OUT_L
WIDTH = AOFF + CHUNK * NUM_CLASS

# pp layout: [ps: BT*64][psT: B][bS: OUT_L (rows 0:65)]
PSOFF = 0
PTOFF = BT * NUM_CLASS
BSOFF = PTOFF + B
PPW = BSOFF + OUT_L

F32 = mybir.dt.float32
F32R = mybir.dt.float32r
BF16 = mybir.dt.bfloat16
F16 = mybir.dt.float16
X = mybir.AxisListType.X

# Stream dtype for the big matmul operands. The PE upconverts 2-byte floats
# to FP22 internally, so fp16 matches float32r's multiply precision while
# halving DMA bytes; only the fp16 input rounding (2^-11) adds error.
#   f32r: ~1.6e-4 rel err, DMA-bound (~77us)
#   f16:  ~5e-4 rel err, PE-bound (~65us)   <- default
#   bf16: ~2.4e-3 rel err (no reason to use; f16 is same speed)
_KDT = os.environ.get("KDT", "f16")
SDT = {"f16": F16, "bf16": BF16, "f32r": F32R}[_KDT]
STREAM_2B = SDT in (F16, BF16)

_cache = {}
# test.py reads this after a traced run for HW exec time
last_results = None


def _build():
    key = f"nc_{_KDT}"
    if key in _cache:
        return _cache[key]
    nc = bacc.Bacc(
        bass.get_trn_type() or "TRN2",
        target_bir_lowering=False,
        debug=False,
        num_devices=NCORES,
    )

    xw = nc.dram_tensor("xw", [NCHUNK, P, WIDTH], SDT, kind="ExternalInput")
    pp = nc.dram_tensor("pp", [P, PPW], F32R, kind="ExternalInput")
    out = nc.dram_tensor("out", [B, OUT_L], F32, kind="ExternalOutput")

    with TileContext(nc) as tc:
        with (
            tc.tile_pool(name="xwp", bufs=1) as xwpool,
            tc.tile_pool(name="sml", bufs=1) as spool,
            tc.tile_pool(name="tl", bufs=1) as tpool,
            tc.tile_pool(name="op", bufs=1) as opool,
            tc.tile_pool(name="dr", bufs=1, space="DRAM") as dpool,
            tc.tile_pool(name="ps", bufs=1, space="PSUM") as ppool,
        ):
            # ---- small inputs (one ACT-ring DMA) ----
            pp_sb = spool.tile([P, PPW], F32R)
            nc.scalar.dma_start(out=pp_sb, in_=pp[:, :])
            ps_sb = pp_sb[:, PSOFF : PSOFF + BT * NUM_CLASS].bitcast(F32)
            psT_sb = pp_sb[:NUM_CLASS, PTOFF : PTOFF + B].bitcast(F32)
            bS_sb = pp_sb[: NUM_CLASS + 1, BSOFF : BSOFF + OUT_L]

            # ---- top-2 threshold per batch row (DVE, alongside matmuls) ----
            m2col = spool.tile([P, BT], F32)
            for bt in range(BT):
                pt = ps_sb[:, bt * NUM_CLASS : (bt + 1) * NUM_CLASS]
                m1 = spool.tile([P, 1], F32, tag=f"m1_{bt}")
                nc.vector.reduce_max(out=m1, in_=pt, axis=X)
                negmask = spool.tile([P, NUM_CLASS], F32, tag=f"nm_{bt}")
                # (pt >= m1) * -1e30  -> additive mask that kills the max
                nc.vector.tensor_scalar(
                    out=negmask,
                    in0=pt,
                    scalar1=m1,
                    scalar2=-1.0e30,
                    op0=mybir.AluOpType.is_ge,
                    op1=mybir.AluOpType.mult,
                )
                p2 = spool.tile([P, NUM_CLASS], F32, tag=f"p2_{bt}")
                nc.vector.tensor_tensor(
                    out=p2, in0=pt, in1=negmask, op=mybir.AluOpType.add
                )
                nc.vector.reduce_max(out=m2col[:, bt : bt + 1], in_=p2, axis=X)

            # ---- PSUM accumulators ----
            mps = [
                ppool.tile([P, OUT_L], F32, tag=f"main{bt}", name=f"main{bt}")
                for bt in range(BT)
            ]
            # bf16: G accumulates as two concurrent column-tiles of one PSUM
            # bank (even k -> rows 0:64, odd k -> rows 64:128); the fp32r
            # matmul path does not support tile_position, so it runs unpacked.
            G_PACK = STREAM_2B
            gt_rows = 2 * NUM_CLASS if G_PACK else NUM_CLASS
            gt_ps = ppool.tile([gt_rows, B], F32, tag="gt", name="gt_ps")

            # ---- main streaming loop over K chunks (one DMA per chunk,
            # alternating HWDGE rings) ----
            # 2-byte streams are PE-bound: ungated (PE never outruns the
            # DMA after chunk 0); fp32r is DMA-bound and needs a backlog
            GATE = 0 if STREAM_2B else 3
            first_mm = None
            for c in range(NCHUNK):
                xwc = xwpool.tile([P, WIDTH], SDT, tag=f"xwc{c}", name=f"xwc{c}")
                dma_eng = nc.sync if c % 2 == 0 else nc.scalar
                xwc_dma = dma_eng.dma_start(out=xwc, in_=xw[c])
                if c == GATE and first_mm is not None:
                    add_dep_helper(
                        first_mm.ins, xwc_dma.ins,
                        reason="gate PE start on a buffered chunk backlog",
                    )
                for kk in range(CHUNK):
                    k = c * CHUNK + kk
                    xk = xwc[:, XOFF + kk * B : XOFF + (kk + 1) * B]
                    wk = xwc[:, WOFF + kk * OUT_L : WOFF + (kk + 1) * OUT_L]
                    ak = xwc[:, AOFF + kk * NUM_CLASS : AOFF + (kk + 1) * NUM_CLASS]
                    def _mains(xk=xk, wk=wk, k=k):
                        global_first = None
                        for bt in range(BT):
                            mm = nc.tensor.matmul(
                                mps[bt],
                                lhsT=xk[:, bt * P : (bt + 1) * P],
                                rhs=wk,
                                start=(k == 0),
                                stop=False,
                            )
                            if global_first is None:
                                global_first = mm
                        return global_first
                    if k < KT - 2:
                        mm = _mains()
                        if first_mm is None:
                            first_mm = mm
                    if G_PACK:
                        half = k % 2
                        nc.tensor.matmul(
                            gt_ps[half * NUM_CLASS : (half + 1) * NUM_CLASS, :],
                            lhsT=ak,
                            rhs=xk,
                            start=(k == half),
                            stop=(k == KT - 2 + half),
                            tile_position=(0, half * NUM_CLASS),
                        )
                    else:
                        nc.tensor.matmul(
                            gt_ps,
                            lhsT=ak,
                            rhs=xk,
                            start=(k == 0),
                            stop=(k == KT - 1),
                        )
                    if k >= KT - 2:
                        # last two k: G first (above), mains after, so the
                        # DVE lora chain overlaps the final main matmuls
                        _mains()
            # ---- LoRA tail ----
            # threshold shuffle on the GPSIMD (SWDGE) path: concurrent with
            # both HWDGE rings, so nothing queues behind the chunk stream.
            # partition->free [128, BT] -> flat [B] via a DRAM bounce, then
            # broadcast-read across 64 partitions (step-0 source dim).
            m2d = dpool.tile([BT, P], F32)
            nc.gpsimd.dma_start(out=m2d.rearrange("bt p -> p bt"), in_=m2col[:, :])
            thr_sb = spool.tile([NUM_CLASS, B], F32)
            nc.gpsimd.dma_start(
                out=thr_sb,
                in_=m2d.rearrange("bt p -> (bt p)")[None, :].broadcast_to(
                    [NUM_CLASS, B]
                ),
            )
            thr2 = tpool.tile([NUM_CLASS, B], F32)
            nc.vector.tensor_copy(out=thr2, in_=thr_sb)
            psT2 = tpool.tile([NUM_CLASS, B], F32)
            nc.vector.tensor_copy(out=psT2, in_=psT_sb)
            maskT = tpool.tile([NUM_CLASS, B], F32)
            nc.vector.tensor_tensor(
                out=maskT, in0=psT2, in1=thr2, op=mybir.AluOpType.is_ge
            )
            gts = tpool.tile([NUM_CLASS, B], F32)
            if G_PACK:
                gt0 = tpool.tile([NUM_CLASS, B], F32)
                nc.vector.tensor_copy(out=gt0, in_=gt_ps[0:NUM_CLASS, :])
                nc.vector.tensor_tensor(
                    out=gts,
                    in0=gt0,
                    in1=gt_ps[NUM_CLASS : 2 * NUM_CLASS, :],
                    op=mybir.AluOpType.add,
                )
            else:
                nc.vector.tensor_copy(out=gts, in_=gt_ps)
            ht = tpool.tile([NUM_CLASS + 1, B], F32R)
            # ones row via in0*0+1 (Memset can't write float32r)
            nc.vector.tensor_scalar(
                out=ht[NUM_CLASS : NUM_CLASS + 1, :],
                in0=thr2[0:1, :],
                scalar1=0.0,
                scalar2=1.0,
                op0=mybir.AluOpType.mult,
                op1=mybir.AluOpType.add,
            )
            nc.vector.tensor_tensor(
                out=ht[0:NUM_CLASS, :], in0=gts, in1=maskT,
                op=mybir.AluOpType.mult,
            )
            for bt in range(BT):
                nc.tensor.matmul(
                    mps[bt],
                    lhsT=ht[:, bt * P : (bt + 1) * P],
                    rhs=bS_sb,
                    start=False,
                    stop=True,
                )

            # ---- epilogue: PSUM -> SBUF -> DRAM, pipelined per tile ----
            o_all = opool.tile([P, BT * OUT_L], F32)
            for bt in range(BT):
                nc.vector.tensor_copy(
                    out=o_all[:, bt * OUT_L : (bt + 1) * OUT_L], in_=mps[bt]
                )
                nc.sync.dma_start(
                    out=out[bt * P : (bt + 1) * P, :],
                    in_=o_all[:, bt * OUT_L : (bt + 1) * OUT_L],
                )

    nc.finalize()
    _cache[key] = nc
    return nc


def _pack_inputs(x, pseudo_index, weight, bias, lora_A, lora_B):
    """Build the interleaved per-core xw buffers + replicated small inputs."""
    xT = np.ascontiguousarray(x.T)                   # [IN, B]
    aT = np.ascontiguousarray(lora_A[:NUM_CLASS].T)  # [IN, 64]

    # [c, kk, p, d] -> [c, p, kk, d]
    x4 = xT.reshape(NCHUNK, CHUNK, P, B).transpose(0, 2, 1, 3)
    a4 = aT.reshape(NCHUNK, CHUNK, P, NUM_CLASS).transpose(0, 2, 1, 3)

    pp_base = np.zeros((P, PPW), dtype=np.float32)
    pp_base[:, PSOFF : PSOFF + BT * NUM_CLASS] = (
        pseudo_index.reshape(BT, P, NUM_CLASS)
        .transpose(1, 0, 2)
        .reshape(P, BT * NUM_CLASS)
    )
    pp_base[:NUM_CLASS, PTOFF : PTOFF + B] = pseudo_index.T

    in_maps = []
    for i in range(NCORES):
        o0 = i * OUT_L
        wTi = weight[o0 : o0 + OUT_L].T              # [IN, OUT_L] (view)
        w4 = wTi.reshape(NCHUNK, CHUNK, P, OUT_L).transpose(0, 2, 1, 3)
        if SDT == F16:
            np_sdt = np.float16
        elif SDT == BF16:
            import ml_dtypes

            np_sdt = ml_dtypes.bfloat16
        else:
            np_sdt = np.float32
        xwi = np.empty((NCHUNK, P, WIDTH), dtype=np_sdt)
        xwi[:, :, XOFF:WOFF] = x4.reshape(NCHUNK, P, CHUNK * B)
        xwi[:, :, WOFF:AOFF] = w4.reshape(NCHUNK, P, CHUNK * OUT_L)
        xwi[:, :, AOFF:WIDTH] = a4.reshape(NCHUNK, P, CHUNK * NUM_CLASS)
        ppi = pp_base.copy()
        ppi[:NUM_CLASS, BSOFF : BSOFF + OUT_L] = (
            16.0 * lora_B[o0 : o0 + OUT_L, :NUM_CLASS].T
        )
        ppi[NUM_CLASS, BSOFF : BSOFF + OUT_L] = 2.0 * bias[o0 : o0 + OUT_L]
        in_maps.append({"xw": xwi, "pp": ppi})
    return in_maps


def kernel(x, pseudo_index, weight, bias, lora_A, lora_B):
    global last_results
    x = np.ascontiguousarray(np.asarray(x, dtype=np.float32))
    pseudo_index = np.ascontiguousarray(np.asarray(pseudo_index, dtype=np.float32))
    weight = np.asarray(weight, dtype=np.float32)
    bias = np.asarray(bias, dtype=np.float32)
    lora_A = np.asarray(lora_A, dtype=np.float32)
    lora_B = np.asarray(lora_B, dtype=np.float32)

    nc = _build()
    in_maps = _pack_inputs(x, pseudo_index, weight, bias, lora_A, lora_B)
    res = run_bass_kernel_spmd(nc, in_maps, list(range(NCORES)))
    last_results = res
    return np.hstack([res.results[i]["out"] for i in range(NCORES)])
